# revision 1
# baseline (speedup 1.0000x reference)
"""LMMD (DSAN local MMD) loss on 8 Trainium2 NeuronCores.

Math (reference):
    X = concat(source, target)                    # [N=4096, D=1024]
    l2[i,j] = max(|x_i|^2 + |x_j|^2 - 2 x_i.x_j, 0)
    bw      = sum(l2) / (N^2 - N) / 4
    K       = sum_q exp(-l2 / (bw * 2^q)),  q = 0..4
    loss    = sum_c v_c^T K v_c / 12,  V = [s_norm; -t_norm]  (rank-12 weights)

Device decomposition (row-sharded, transposed tiles):
    Core c owns i-block I_c = [512c, 512(c+1)).  It computes, for every
    j-tile (128 rows of the full 4096), the gram block G[j, i in I_c] via
    PE matmuls (lhsT = X^T[:, j-block] streamed bf16 weights, rhs = own
    X^T columns, contraction over D), then
        F_q[j,i] = exp(2 c_q G - c_q sq_j)     (c_q = 1/(bw 2^q))
    on ACT (q=4 and q=0 directly; F_3 = F_4^2, F_2 = F_3^2, F_1 = F_2^2 on
    DVE), and reduces with a second PE matmul into PSUM accumulators
        R_q[cls, i] += V[j, cls]^T F_q[j, i].
    Host applies alpha_q[i] = exp(-c_q sq_i) and the final V-contraction:
        loss = 1/12 sum_q sum_i alpha_q[i] sum_cls V[i,cls] R_q[cls,i].
    bw is computed analytically on host: sum(l2) = 2N*sum(sq) - 2|colsum|^2
    (the relu clamp only affects the diagonal at ~1e-10 relative).
"""

import numpy as np
import ml_dtypes

import concourse.bass as bass
from concourse import bacc
import concourse.mybir as mybir
import concourse.tile as tile
from concourse.bass_utils import run_bass_kernel_spmd

B = 2048
D = 1024
C = 12
NCORES = 8
N = 2 * B                 # 4096 total samples
IPC = N // NCORES         # 512 own columns (i) per core
NJT = N // 128            # 32 j-tiles
NKC = D // 128            # 8 contraction chunks
JG = 4                    # j-groups (weight DMA granularity)
JPG = NJT // JG           # 8 j-tiles per group
NQ = 5                    # kernels in the RBF mixture
DELAY = 2                 # j-tiles of software pipelining for weighted MMs

_BUILT = None             # (nc,) cache — program is input-independent


def _build_program():
    fp32 = mybir.dt.float32
    f32r = mybir.dt.float32r
    bf16 = mybir.dt.bfloat16
    Exp = mybir.ActivationFunctionType.Exp

    nc = bacc.Bacc()
    xtb = nc.declare_dram_parameter("xtb", [D, N], bf16, isOutput=False)
    own = nc.declare_dram_parameter("own", [D, IPC], bf16, isOutput=False)
    vt = nc.declare_dram_parameter("vt", [128, NJT * C], f32r, isOutput=False)
    qsq = nc.declare_dram_parameter("qsq", [128, NJT * NQ], fp32, isOutput=False)
    scl = nc.declare_dram_parameter("scl", [128, NQ], fp32, isOutput=False)
    rout = nc.declare_dram_parameter("r_out", [NQ, C, IPC], fp32, isOutput=True)

    with tile.TileContext(nc) as tc:
        with (
            tc.tile_pool(name="singles", bufs=1) as singles,
            tc.tile_pool(name="wpool", bufs=2) as wpool,
            tc.tile_pool(name="epool", bufs=3) as epool,
            tc.tile_pool(name="ostage", bufs=1) as ostage,
            tc.tile_pool(name="gpsum", bufs=3, space="PSUM") as gpsum,
            tc.tile_pool(name="rqpsum", bufs=1, space="PSUM") as rqpsum,
        ):
            own_sb = singles.tile([128, NKC * IPC], bf16)
            for k in range(NKC):
                nc.sync.dma_start(
                    out=own_sb[:, k * IPC : (k + 1) * IPC],
                    in_=own[k * 128 : (k + 1) * 128, :],
                )
            vt_sb = singles.tile([128, NJT * C], f32r)
            nc.sync.dma_start(out=vt_sb[:], in_=vt[:])
            qsq_sb = singles.tile([128, NJT * NQ], fp32)
            nc.sync.dma_start(out=qsq_sb[:], in_=qsq[:])
            scl_sb = singles.tile([128, NQ], fp32)
            nc.sync.dma_start(out=scl_sb[:], in_=scl[:])
            # Stage small tiles through DVE so consumers wait on one
            # semaphore instead of the DMA queue fan-out (walrus caps the
            # per-instruction sync-wait count).
            vt_s = singles.tile([128, NJT * C], f32r)
            nc.vector.tensor_copy(vt_s, vt_sb)
            qsq_s = singles.tile([128, NJT * NQ], fp32)
            nc.vector.tensor_copy(qsq_s, qsq_sb)
            scl_s = singles.tile([128, NQ], fp32)
            nc.vector.tensor_copy(scl_s, scl_sb)
            # Dummy ACT op: absorbs the DVE wait (walrus allows a single
            # sync-wait slot per Activation), so loop Exp ops only ever
            # need the PE wait.
            warm = singles.tile([128, NQ], fp32)
            nc.scalar.activation(warm, scl_s, Exp)

            rq = [rqpsum.tile([C, IPC], fp32, tag=f"rq{q}", name=f"rq{q}") for q in range(NQ)]

            def emit_weighted(jt, es):
                lhs = vt_s[:, jt * C : (jt + 1) * C]
                for q in range(NQ):
                    nc.tensor.matmul(
                        rq[q],
                        lhsT=lhs,
                        rhs=es[q],
                        start=(jt == 0),
                        stop=(jt == NJT - 1),
                    )

            pending = []
            for jg in range(JG):
                w = [wpool.tile([128, JPG * 128], bf16, tag=f"wk{k}", name=f"w{k}") for k in range(NKC)]
                for k in range(NKC):
                    nc.sync.dma_start(
                        out=w[k],
                        in_=xtb[k * 128 : (k + 1) * 128, jg * JPG * 128 : (jg + 1) * JPG * 128],
                    )
                for jl in range(JPG):
                    jt = jg * JPG + jl
                    g = gpsum.tile([128, IPC], fp32)
                    for k in range(NKC):
                        nc.tensor.matmul(
                            g,
                            lhsT=w[k][:, jl * 128 : (jl + 1) * 128],
                            rhs=own_sb[:, k * IPC : (k + 1) * IPC],
                            start=(k == 0),
                            stop=(k == NKC - 1),
                        )
                    e4 = epool.tile([128, IPC], f32r, bufs=NJT)
                    e3 = epool.tile([128, IPC], f32r)
                    e2 = epool.tile([128, IPC], f32r)
                    e1 = epool.tile([128, IPC], f32r)
                    e0 = epool.tile([128, IPC], f32r)
                    nc.scalar.activation(
                        e4, g, Exp,
                        bias=qsq_s[:, jt * NQ + 4 : jt * NQ + 5],
                        scale=scl_s[:, 4:5],
                    )
                    nc.scalar.activation(
                        e0, g, Exp,
                        bias=qsq_s[:, jt * NQ : jt * NQ + 1],
                        scale=scl_s[:, 0:1],
                    )
                    nc.vector.tensor_mul(e3, e4, e4)
                    nc.vector.tensor_mul(e2, e3, e3)
                    nc.vector.tensor_mul(e1, e2, e2)
                    pending.append((jt, [e0, e1, e2, e3, e4]))
                    if len(pending) > DELAY:
                        emit_weighted(*pending.pop(0))
            for item in pending:
                emit_weighted(*item)

            for q in range(NQ):
                stg = ostage.tile([C, IPC], fp32, tag=f"st{q}", name=f"st{q}")
                nc.vector.tensor_copy(stg, rq[q])
                nc.sync.dma_start(out=rout[q], in_=stg)

    nc.compile()
    return nc


def _prep(source, target, source_label, target_logits):
    X = np.concatenate([np.asarray(source), np.asarray(target)], axis=0)
    X64 = X.astype(np.float64)
    sq = np.einsum("nd,nd->n", X64, X64)
    colsum = X64.sum(axis=0)
    sum_l2 = 2.0 * N * sq.sum() - 2.0 * (colsum @ colsum)
    bw = sum_l2 / (N * N - N) / (2.0 ** (5 // 2))
    cq = np.array([1.0 / (bw * 2.0**q) for q in range(NQ)])  # [5]

    sl = np.asarray(source_label, np.float64)
    tl = np.asarray(target_logits, np.float64)
    ssum = sl.sum(0)
    s_norm = np.where(ssum > 0, sl / np.where(ssum > 0, ssum, 1.0), 0.0)
    tsum = tl.sum(0)
    t_norm = np.where(tsum > 0, tl / np.where(tsum > 0, tsum, 1.0), 0.0)
    s_pres = np.zeros(C)
    np.add.at(s_pres, sl.argmax(1), 1.0)
    t_pres = np.zeros(C)
    np.add.at(t_pres, tl.argmax(1), 1.0)
    common = ((s_pres > 0) & (t_pres > 0)).astype(np.float64)
    V = np.concatenate([s_norm * common, -t_norm * common], axis=0)  # [N, C]

    xtb = np.ascontiguousarray(X.T).astype(ml_dtypes.bfloat16)  # [D, N]
    vt = np.ascontiguousarray(
        V.reshape(NJT, 128, C).transpose(1, 0, 2).reshape(128, NJT * C)
    ).astype(np.float32)
    # qsq[p, jt*5+q] = -c_q * sq[jt*128 + p]
    sqt = sq.reshape(NJT, 128)
    qsq = np.ascontiguousarray(
        (-cq[None, None, :] * sqt[:, :, None]).transpose(1, 0, 2).reshape(128, NJT * NQ)
    ).astype(np.float32)
    scl = np.broadcast_to((2.0 * cq).astype(np.float32), (128, NQ)).copy()
    return X, sq, cq, V, xtb, vt, qsq, scl


def _postprocess(results, sq, cq, V):
    # loss = 1/12 sum_q sum_i alpha_q[i] * (sum_cls V[i,cls] R_q[cls,i])
    loss = 0.0
    for c in range(NCORES):
        r = np.asarray(results[c]["r_out"], np.float64)  # [5, 12, 512]
        i0 = c * IPC
        Vc = V[i0 : i0 + IPC]                  # [512, 12]
        alpha = np.exp(-np.outer(cq, sq[i0 : i0 + IPC]))  # [5, 512]
        loss += np.einsum("qi,ic,qci->", alpha, Vc, r)
    return loss / C


def _run(in_maps, trace=False, **kw):
    global _BUILT
    if _BUILT is None:
        _BUILT = _build_program()
    return run_bass_kernel_spmd(_BUILT, in_maps, list(range(NCORES)), trace=trace, **kw)


def kernel(source, target, source_label, target_logits, _trace=False, _ret_bkr=False):
    X, sq, cq, V, xtb, vt, qsq, scl = _prep(source, target, source_label, target_logits)
    in_maps = []
    for c in range(NCORES):
        own = np.ascontiguousarray(xtb[:, c * IPC : (c + 1) * IPC])
        in_maps.append(
            {"xtb": xtb, "own": own, "vt": vt, "qsq": qsq, "scl": scl}
        )
    bkr = _run(in_maps, trace=_trace)
    loss = _postprocess(bkr.results, sq, cq, V)
    out = np.float32(loss)
    if _ret_bkr:
        return out, bkr
    return out



# revision 5
# speedup vs baseline: 2.7730x; 2.7730x over previous
"""LMMD (DSAN local MMD) loss on 8 Trainium2 NeuronCores — triangle + fp8 V2.

Math (reference):
    X = concat(source, target)                    # [N=4096, D=1024]
    l2[i,j] = max(|x_i|^2 + |x_j|^2 - 2 x_i.x_j, 0)
    bw      = sum(l2) / (N^2 - N) / 4
    K       = sum_q exp(-l2 / (bw * 2^q)),  q = 0..4
    loss    = sum_c v_c^T K v_c / 12,  V = [s_norm; -t_norm]  (rank-12 weights)

V2 design (vs row-sharded baseline):
  * Triangle: K and the rank-12 weight matrix are symmetric, so each
    unordered tile-pair {u,v} of the 32x32 (j,i) 128-tile grid is computed
    once (weight 2 off-diagonal via vt2 = 2V, weight 1 diagonal via vt1 = V).
    A uniform edge-orientation makes the 8 per-core programs IDENTICAL:
    core c owns i-tiles [4c..4c+3]; its j-slot sequence is its own 4 tiles
    (suffix spans 512/384/256/128) then, per foreign core d, 4 slots with
    fixed i-column halves [0:256],[0:256],[256:512],[256:512], where the
    host permutes j-tiles (d>c: natural order, d<c: swapped halves) so that
    globally every pair is covered exactly once. 66 tile-jobs per core
    (vs 128 for the full row-block) — verified by brute force.
  * Gram in fp8e4m3 with MatmulPerfMode.DoubleRow: 4 matmuls per slot,
    each contracting 2 of the 8 128-deep k-chunks at 0.5 cycles/row.
    Host-side fp64 sq/bias tables keep l2 = sq_j + sq_i - 2G accurate;
    measured end-to-end rel err ~4e-3 (gate 2e-2).
  * es in bf16: e4 = exp(2 c4 G - c4 sq_j) on ACT (per-partition bias),
    then e3..e0 by DVE squaring (2x_1p mode), batched across a group's 4
    slots in one wide SBUF tile to amortize per-op overhead.
  * Weighted reduce: R_q[cls,i] += vt^T es_q per 128-col block with
    per-block start/stop accumulation in PSUM; host applies
    alpha_q[i] = exp(-c_q sq_i) and the final V-contraction.
  * DMA: xtb staged host-side as [128, 8k, 4096] fp8 so one dma_start
    moves a whole 512-column j-group (the shared HWDGE device costs
    625ns per dma_start — count matters).
"""

import numpy as np
import ml_dtypes

import concourse.bass as bass
from concourse import bacc
import concourse.mybir as mybir
import concourse.tile as tile
from concourse.bass_utils import run_bass_kernel_spmd

B = 2048
D = 1024
C = 12
NCORES = 8
N = 2 * B                 # 4096 total samples
IPC = N // NCORES         # 512 own columns (i) per core
NT = N // 128             # 32 j-tiles
NKC = D // 128            # 8 contraction chunks
NKP = NKC // 2            # 4 DoubleRow chunk-pairs
NQ = 5                    # kernels in the RBF mixture
NGROUPS = 8               # slot groups: own + 7 foreign cores

# (block_off, block_end) in 128-col i-blocks, per slot-in-group
OWN_SPANS = [(0, 4), (1, 4), (2, 4), (3, 4)]
FOREIGN_PAT = [(0, 2), (0, 2), (2, 4), (2, 4)]
# last slot index writing each 128-col block (own slots 0-3, foreign 4..31)
BLOCK_LAST = {0: 29, 1: 29, 2: 31, 3: 31}
NWARM = 8                 # PE p-state warm matmuls during startup DMA

F8NP = ml_dtypes.float8_e4m3
BFNP = ml_dtypes.bfloat16

_BUILT = None             # program is input-independent


def _slot_geom(g, sl):
    if g == 0:
        off, end = OWN_SPANS[sl]
    else:
        off, end = FOREIGN_PAT[sl]
    return off, end


def _ebase(g, sl):
    # column base of slot sl inside the group's batched e tiles
    bases = [0]
    for s in range(1, 4):
        o, e = _slot_geom(g, s - 1)
        bases.append(bases[-1] + (e - o) * 128)
    return bases[sl]


def _build_program():
    fp32 = mybir.dt.float32
    bf16 = mybir.dt.bfloat16
    f8 = mybir.dt.float8e4
    Exp = mybir.ActivationFunctionType.Exp
    DR = mybir.MatmulPerfMode.DoubleRow

    nc = bacc.Bacc()
    # host-pretransposed: xtb[p, k, s*128+j] = X[jseq[s]*128+j, k*128+p]
    xtb = nc.declare_dram_parameter("xtb", [128, NKC, NT * 128], f8, isOutput=False)
    vt2 = nc.declare_dram_parameter("vt2", [128, NT * C], bf16, isOutput=False)
    vt1 = nc.declare_dram_parameter("vt1", [128, 4 * C], bf16, isOutput=False)
    qsq = nc.declare_dram_parameter("qsq", [128, NT], fp32, isOutput=False)
    scl = nc.declare_dram_parameter("scl", [128, 2], fp32, isOutput=False)
    rout = nc.declare_dram_parameter("r_out", [C, NQ * IPC], fp32, isOutput=True)

    with tile.TileContext(nc) as tc:
        with (
            tc.tile_pool(name="singles", bufs=1) as singles,
            tc.tile_pool(name="wpool", bufs=2) as wpool,
            tc.tile_pool(name="epool", bufs=2) as epool,
            tc.tile_pool(name="ostage", bufs=1) as ostage,
            tc.tile_pool(name="gpsum", bufs=3, space="PSUM") as gpsum,
            tc.tile_pool(name="rqpsum", bufs=1, space="PSUM") as rqpsum,
        ):
            vt2_sb = singles.tile([128, NT * C], bf16)
            nc.sync.dma_start(out=vt2_sb, in_=vt2[:])
            vt1_sb = singles.tile([128, 4 * C], bf16)
            nc.sync.dma_start(out=vt1_sb, in_=vt1[:])
            qsq_sb = singles.tile([128, NT], fp32)
            nc.sync.dma_start(out=qsq_sb, in_=qsq[:])
            scl_sb = singles.tile([128, 2], fp32)
            nc.sync.dma_start(out=scl_sb, in_=scl[:])
            # Stage small tiles through DVE so consumers wait on one
            # semaphore instead of the DMA queue fan-out (walrus caps the
            # per-instruction sync-wait count).
            vt2_s = singles.tile([128, NT * C], bf16)
            nc.vector.tensor_copy(vt2_s, vt2_sb)
            vt1_s = singles.tile([128, 4 * C], bf16)
            nc.vector.tensor_copy(vt1_s, vt1_sb)
            qsq_s = singles.tile([128, NT], fp32)
            nc.vector.tensor_copy(qsq_s, qsq_sb)
            scl_s = singles.tile([128, 2], fp32)
            nc.vector.tensor_copy(scl_s, scl_sb)
            # Dummy ACT op: loads the Exp table early and absorbs the DVE
            # wait so loop Exp ops only ever need the PE wait.
            warm = singles.tile([128, 2], fp32)
            nc.scalar.activation(warm, scl_s, Exp)

            rq = [rqpsum.tile([C, IPC], fp32, tag=f"rq{q}", name=f"rq{q}") for q in range(NQ)]

            # PE p-state warm-up: harmless matmuls on staged vt2 while the
            # first weight DMAs stream (results never read).
            for wi in range(NWARM):
                wt = gpsum.tile([128, IPC], fp32, tag="g", name=f"wm{wi}")
                nc.tensor.matmul(
                    wt[0:C, 0 : 4 * C],
                    lhsT=vt2_s[:, 0:C],
                    rhs=vt2_s[:, 0 : 4 * C],
                    start=True,
                    stop=True,
                )

            def emit_weighted(g, es):
                # PSUM zero-region semantics: exactly one start (first matmul
                # into the bank — hw zeroes the whole bank) and one stop (last
                # matmul into the bank) per rq tile.
                for sl in range(4):
                    slot = 4 * g + sl
                    off, end = _slot_geom(g, sl)
                    eb = _ebase(g, sl)
                    for q in range(NQ):
                        for b in range(off, end):
                            if g == 0 and b == sl:
                                lhs = vt1_s[:, sl * C : (sl + 1) * C]
                            else:
                                lhs = vt2_s[:, slot * C : (slot + 1) * C]
                            col = eb + (b - off) * 128
                            nc.tensor.matmul(
                                rq[q][:, b * 128 : (b + 1) * 128],
                                lhsT=lhs,
                                rhs=es[q][:, col : col + 128],
                                start=(g == 0 and sl == 0 and b == 0),
                                stop=(g == NGROUPS - 1 and sl == 3 and b == 3),
                            )

            pending = None
            for g in range(NGROUPS):
                if g == 0:
                    wsrc = own_sb
                else:
                    wg = wpool.tile([128, NKC, 512], f8, tag="wg", name=f"w{g}")
                    nc.sync.dma_start(out=wg, in_=xtb[:, :, g * 512 : (g + 1) * 512])
                    wsrc = wg
                gw = 1280 if g == 0 else 1024  # total e-batch width
                es = {q: epool.tile([128, 1280], bf16, tag=f"e{q}", name=f"e{q}g{g}") for q in range(NQ)}
                for sl in range(4):
                    slot = 4 * g + sl
                    off, end = _slot_geom(g, sl)
                    span = (end - off) * 128
                    eb = _ebase(g, sl)
                    gt = gpsum.tile([128, IPC], fp32, tag="g", name=f"g{slot}")
                    for m in range(NKP):
                        nc.tensor.matmul(
                            gt[:, 0:span],
                            lhsT=wsrc[:, 2 * m : 2 * m + 2, sl * 128 : (sl + 1) * 128],
                            rhs=own_sb[:, 2 * m : 2 * m + 2, off * 128 : end * 128],
                            start=(m == 0),
                            stop=(m == NKP - 1),
                            perf_mode=DR,
                        )
                    nc.scalar.activation(
                        es[4][:, eb : eb + span],
                        gt[:, 0:span],
                        Exp,
                        bias=qsq_s[:, slot : slot + 1],
                        scale=scl_s[:, 0:1],
                    )
                nc.vector.tensor_mul(es[3][:, 0:gw], es[4][:, 0:gw], es[4][:, 0:gw])
                nc.vector.tensor_mul(es[2][:, 0:gw], es[3][:, 0:gw], es[3][:, 0:gw])
                nc.vector.tensor_mul(es[1][:, 0:gw], es[2][:, 0:gw], es[2][:, 0:gw])
                nc.vector.tensor_mul(es[0][:, 0:gw], es[1][:, 0:gw], es[1][:, 0:gw])
                if pending is not None:
                    emit_weighted(*pending)
                pending = (g, es)
            emit_weighted(*pending)

            stg = ostage.tile([C, NQ * IPC], fp32)
            for q in range(NQ):
                nc.vector.tensor_copy(stg[:, q * IPC : (q + 1) * IPC], rq[q])
            nc.sync.dma_start(out=rout[:], in_=stg)

    nc.compile()
    return nc


def _jseq(c):
    seq = list(range(4 * c, 4 * c + 4))
    for d in range(NCORES):
        if d == c:
            continue
        if d > c:
            seq += [4 * d, 4 * d + 1, 4 * d + 2, 4 * d + 3]
        else:
            seq += [4 * d + 2, 4 * d + 3, 4 * d, 4 * d + 1]
    return seq


def _prep(source, target, source_label, target_logits):
    X = np.concatenate([np.asarray(source), np.asarray(target)], axis=0)
    X64 = X.astype(np.float64)
    sq = np.einsum("nd,nd->n", X64, X64)
    colsum = X64.sum(axis=0)
    sum_l2 = 2.0 * N * sq.sum() - 2.0 * (colsum @ colsum)
    bw = sum_l2 / (N * N - N) / (2.0 ** (NQ // 2))
    cq = np.array([1.0 / (bw * 2.0**q) for q in range(NQ)])  # [5]

    sl = np.asarray(source_label, np.float64)
    tl = np.asarray(target_logits, np.float64)
    ssum = sl.sum(0)
    s_norm = np.where(ssum > 0, sl / np.where(ssum > 0, ssum, 1.0), 0.0)
    tsum = tl.sum(0)
    t_norm = np.where(tsum > 0, tl / np.where(tsum > 0, tsum, 1.0), 0.0)
    s_pres = np.zeros(C)
    np.add.at(s_pres, sl.argmax(1), 1.0)
    t_pres = np.zeros(C)
    np.add.at(t_pres, tl.argmax(1), 1.0)
    common = ((s_pres > 0) & (t_pres > 0)).astype(np.float64)
    V = np.concatenate([s_norm * common, -t_norm * common], axis=0)  # [N, C]

    # fp8 X^T in [p, k, jcol] layout (global j order; per-core slot perm later)
    X8 = X.astype(F8NP)                                   # [N, D]
    xt8 = np.ascontiguousarray(
        X8.T.reshape(NKC, 128, N).transpose(1, 0, 2)      # [128, 8, N]
    )
    Vb = V.astype(BFNP)
    sqt = sq.reshape(NT, 128)
    return X, sq, cq, V, Vb, xt8, sqt


def _core_inputs(c, cq, Vb, xt8, sqt):
    seq = _jseq(c)
    # xtb: permute j-tiles into slot order
    xtb = np.ascontiguousarray(
        xt8.reshape(128, NKC, NT, 128)[:, :, seq, :].reshape(128, NKC, NT * 128)
    )
    Vt = Vb.astype(np.float64).reshape(NT, 128, C)[seq]   # [NT, 128, C]
    vt2 = np.ascontiguousarray((2.0 * Vt).transpose(1, 0, 2).reshape(128, NT * C)).astype(BFNP)
    vt1 = np.ascontiguousarray(Vt[:4].transpose(1, 0, 2).reshape(128, 4 * C)).astype(BFNP)
    qsq = np.ascontiguousarray((-cq[4] * sqt[seq]).T).astype(np.float32)  # [128, NT]
    scl = np.zeros((128, 2), np.float32)
    scl[:, 0] = 2.0 * cq[4]
    return {"xtb": xtb, "vt2": vt2, "vt1": vt1, "qsq": qsq, "scl": scl}


def _postprocess(results, sq, cq, V):
    # loss = 1/12 sum_q sum_i alpha_q[i] * (sum_cls V[i,cls] R_q[cls,i])
    loss = 0.0
    for c in range(NCORES):
        r = np.asarray(results[c]["r_out"], np.float64).reshape(C, NQ, IPC)
        gi = c * IPC + np.arange(IPC)
        Vc = V[gi]                                        # [512, 12]
        alpha = np.exp(-np.outer(cq, sq[gi]))             # [5, 512]
        loss += np.einsum("qi,ic,cqi->", alpha, Vc, r)
    return loss / C


def _run(in_maps, trace=False, **kw):
    global _BUILT
    if _BUILT is None:
        _BUILT = _build_program()
    return run_bass_kernel_spmd(_BUILT, in_maps, list(range(NCORES)), trace=trace, **kw)


def kernel(source, target, source_label, target_logits, _trace=False, _ret_bkr=False):
    X, sq, cq, V, Vb, xt8, sqt = _prep(source, target, source_label, target_logits)
    in_maps = [_core_inputs(c, cq, Vb, xt8, sqt) for c in range(NCORES)]
    bkr = _run(in_maps, trace=_trace)
    loss = _postprocess(bkr.results, sq, cq, V)
    out = np.float32(loss)
    if _ret_bkr:
        return out, bkr
    return out


# revision 27
# speedup vs baseline: 3.6414x; 1.3132x over previous
"""LMMD (DSAN local MMD) loss on 8 Trainium2 NeuronCores — triangle + fp8 V2.

Math (reference):
    X = concat(source, target)                    # [N=4096, D=1024]
    l2[i,j] = max(|x_i|^2 + |x_j|^2 - 2 x_i.x_j, 0)
    bw      = sum(l2) / (N^2 - N) / 4
    K       = sum_q exp(-l2 / (bw * 2^q)),  q = 0..4
    loss    = sum_c v_c^T K v_c / 12,  V = [s_norm; -t_norm]  (rank-12 weights)

Design (vs full row-sharded baseline):
  * Triangle: K and the rank-12 weight matrix are symmetric, so each
    unordered tile-pair {u,v} of the 32x32 (j,i) 128-tile grid is computed
    once (weight 2 off-diagonal via vt2 = 2V, weight 1 diagonal via vt1 = V).
    A uniform edge-orientation makes the 8 per-core programs IDENTICAL:
    core c owns i-tiles [4c..4c+3]; its j-slot sequence is its own 4 tiles
    (suffix spans 512/384/256/128) then, per foreign core d, 4 slots with
    fixed i-column halves [0:256],[0:256],[256:512],[256:512], where the
    host permutes j-tiles (d>c: natural order, d<c: swapped halves) so that
    globally every pair is covered exactly once. 66 tile-jobs per core
    (vs 128 for the full row-block) — verified by brute force.
  * Gram in fp8e4m3 with MatmulPerfMode.DoubleRow: 4 matmuls per slot,
    each contracting 2 of the 8 128-deep k-chunks at 0.5 cycles/row.
    Host-side fp64 sq/bias tables keep l2 = sq_j + sq_i - 2G accurate;
    measured end-to-end rel err ~4e-3 (gate 2e-2).
  * es in bf16: e4 = exp(2 c4 G - c4 sq_j) on ACT (per-partition bias),
    then e3/e2/e1 by DVE squaring (2x_1p), e0 split Pool/DVE — batched
    across a group's 4 slots in one wide SBUF tile.
  * Weighted reduce FLIPPED: the es block (128j x 128i) is the STATIONARY
    PE operand and the 12-class vt block is the moving tensor, so each
    accumulation matmul streams 12 rows instead of 128 (and fills the PE
    array 128-wide instead of 12-wide — also the right choice on real hw).
    R_q[i, cls] accumulates in PSUM per own 128-col block; host applies
    alpha_q[i] = exp(-c_q sq_i) and the final V-contraction.
  * DMA: xtb staged host-side as [128, 8k, 4096] fp8 so one dma_start
    moves a whole 512-column j-group (the shared HWDGE device costs
    625ns per dma_start — count matters).
"""

import numpy as np
import ml_dtypes

import concourse.bass as bass
from concourse import bacc
import concourse.mybir as mybir
import concourse.tile as tile
from concourse.bass_utils import run_bass_kernel_spmd

B = 2048
D = 1024
C = 12
NCORES = 8
N = 2 * B                 # 4096 total samples
IPC = N // NCORES         # 512 own columns (i) per core
NT = N // 128             # 32 j-tiles
NKC = D // 128            # 8 contraction chunks
NKP = NKC // 2            # 4 DoubleRow chunk-pairs
NQ = 5                    # kernels in the RBF mixture
NGROUPS = 8               # slot groups: own + 7 foreign cores
OWN_G = 0                 # own group first (cheapest start: no weight DMA)
WLAG = 3                  # groups of lag between es production and weighted use

# (block_off, block_end) in 128-col i-blocks, per slot-in-group
OWN_SPANS = [(0, 4), (1, 4), (2, 4), (3, 4)]
FOREIGN_PAT = [(0, 2), (0, 2), (2, 4), (2, 4)]

F8NP = ml_dtypes.float8_e4m3
BFNP = ml_dtypes.bfloat16

_BUILT = None             # program is input-independent


def _slot_geom(g, sl):
    if g == OWN_G:
        off, end = OWN_SPANS[sl]
    else:
        off, end = FOREIGN_PAT[sl]
    return off, end


def _ebase(g, sl):
    # column base of slot sl inside the group's batched e tiles
    bases = [0]
    for s in range(1, 4):
        o, e = _slot_geom(g, s - 1)
        bases.append(bases[-1] + (e - o) * 128)
    return bases[sl]


def _build_program():
    fp32 = mybir.dt.float32
    bf16 = mybir.dt.bfloat16
    f8 = mybir.dt.float8e4
    Exp = mybir.ActivationFunctionType.Exp
    Copy = mybir.ActivationFunctionType.Copy
    DR = mybir.MatmulPerfMode.DoubleRow

    nc = bacc.Bacc()
    # host-pretransposed: xtb[p, k, s*128+j] = X[jseq[s]*128+j, k*128+p]
    xtb = nc.declare_dram_parameter("xtb", [128, NKC, NT * 128], f8, isOutput=False)
    # ftab = qsq[NT] | scl[2];  btab = vt2[NT*C] | vt1[4*C]
    ftab = nc.declare_dram_parameter("ftab", [128, NT + 2], fp32, isOutput=False)
    btab = nc.declare_dram_parameter("btab", [128, NT * C + 4 * C], bf16, isOutput=False)
    # flipped weighted layout: rows = i within own 128-block, cols = (q, block, cls)
    rout = nc.declare_dram_parameter("r_out", [128, NQ * 4 * C], fp32, isOutput=True)

    with tile.TileContext(nc) as tc:
        with (
            tc.tile_pool(name="singles", bufs=1) as singles,
            tc.tile_pool(name="wpool", bufs=3) as wpool,
            tc.tile_pool(name="epool", bufs=5) as epool,
            tc.tile_pool(name="ostage", bufs=1) as ostage,
            tc.tile_pool(name="gpsum", bufs=3, space="PSUM") as gpsum,
            tc.tile_pool(name="rqpsum", bufs=1, space="PSUM") as rqpsum,
        ):
            # own i-columns = slots 0..3 of xtb, first on the serialized DMA
            # path (gram slot 0 starts after the first half); small tables
            # follow in consumer order (exp bias/scale before vt).
            own_sb = singles.tile([128, NKC, IPC], f8)
            nc.sync.dma_start(out=own_sb[:, 0 : NKC // 2, :], in_=xtb[:, 0 : NKC // 2, 0:IPC])
            nc.sync.dma_start(out=own_sb[:, NKC // 2 : NKC, :], in_=xtb[:, NKC // 2 : NKC, 0:IPC])
            ftab_sb = singles.tile([128, NT + 2], fp32)
            nc.sync.dma_start(out=ftab_sb, in_=ftab[:])
            btab_sb = singles.tile([128, NT * C + 4 * C], bf16)
            nc.sync.dma_start(out=btab_sb, in_=btab[:])
            # Stage small tiles through DVE so consumers wait on one
            # semaphore instead of the DMA queue fan-out (walrus caps the
            # per-instruction sync-wait count).
            qsq_s = singles.tile([128, NT], fp32)
            nc.vector.tensor_copy(qsq_s, ftab_sb[:, 0:NT])
            scl_s = singles.tile([128, 2], fp32)
            nc.vector.tensor_copy(scl_s, ftab_sb[:, NT : NT + 2])
            vt2_s = singles.tile([128, NT * C], bf16)
            nc.vector.tensor_copy(vt2_s, btab_sb[:, 0 : NT * C])
            vt1_s = singles.tile([128, 4 * C], bf16)
            nc.vector.tensor_copy(vt1_s, btab_sb[:, NT * C : NT * C + 4 * C])
            # Dummy ACT op: loads the Exp table early and absorbs the DVE
            # wait so loop Exp ops only ever need the PE wait.
            warm = singles.tile([128, 2], fp32)
            nc.scalar.activation(warm, scl_s, Exp)

            # rq[q][i, b*C+cls] accumulates R_q over j for own block b
            rq = [rqpsum.tile([128, 4 * C], fp32, tag=f"rq{q}", name=f"rq{q}") for q in range(NQ)]

            def emit_weighted(g, es):
                # Flipped orientation: es block stationary, vt moving.
                # q-major, q=4 first so PE chases the DVE squaring chain.
                # PSUM zero-region semantics: exactly one start (first matmul
                # into the bank) and one stop (last) per rq tile.
                for q in range(NQ - 1, -1, -1):
                    for sl in range(4):
                        slot = 4 * g + sl
                        off, end = _slot_geom(g, sl)
                        eb = _ebase(g, sl)
                        for b in range(off, end):
                            if g == OWN_G and b == sl:
                                vtb = vt1_s[:, sl * C : (sl + 1) * C]
                            else:
                                vtb = vt2_s[:, slot * C : (slot + 1) * C]
                            col = eb + (b - off) * 128
                            nc.tensor.matmul(
                                rq[q][:, b * C : (b + 1) * C],
                                lhsT=es[q][:, col : col + 128],
                                rhs=vtb,
                                start=(g == 0 and sl == 0 and b == 0),
                                stop=(g == NGROUPS - 1 and sl == 3 and b == 3),
                            )

            pending = []
            for g in range(NGROUPS):
                if g == OWN_G:
                    wsrc = own_sb
                else:
                    wg = wpool.tile([128, NKC, 512], f8, tag="wg", name=f"w{g}")
                    src0 = g * 512
                    nc.sync.dma_start(out=wg, in_=xtb[:, :, src0 : src0 + 512])
                    wsrc = wg
                gw = 1280 if g == OWN_G else 1024  # total e-batch width
                es = {q: epool.tile([128, 1280], bf16, tag=f"e{q}", name=f"e{q}g{g}") for q in range(NQ)}
                for sl in range(4):
                    slot = 4 * g + sl
                    off, end = _slot_geom(g, sl)
                    span = (end - off) * 128
                    eb = _ebase(g, sl)
                    gt = gpsum.tile([128, IPC], fp32, tag="g", name=f"g{slot}")
                    for m in range(NKP):
                        nc.tensor.matmul(
                            gt[:, 0:span],
                            lhsT=wsrc[:, 2 * m : 2 * m + 2, sl * 128 : (sl + 1) * 128],
                            rhs=own_sb[:, 2 * m : 2 * m + 2, off * 128 : end * 128],
                            start=(m == 0),
                            stop=(m == NKP - 1),
                            perf_mode=DR,
                        )
                    nc.scalar.activation(
                        es[4][:, eb : eb + span],
                        gt[:, 0:span],
                        Exp,
                        bias=qsq_s[:, slot : slot + 1],
                        scale=scl_s[:, 0:1],
                    )
                # squaring chain: e3/e2/e1 on DVE (2x_1p), e0 split between
                # the idle Pool engine and DVE
                nc.vector.tensor_mul(es[3][:, 0:gw], es[4][:, 0:gw], es[4][:, 0:gw])
                nc.vector.tensor_mul(es[2][:, 0:gw], es[3][:, 0:gw], es[3][:, 0:gw])
                nc.vector.tensor_mul(es[1][:, 0:gw], es[2][:, 0:gw], es[2][:, 0:gw])
                hw_ = 21 * gw // 32
                nc.gpsimd.tensor_mul(es[0][:, 0:hw_], es[1][:, 0:hw_], es[1][:, 0:hw_])
                nc.vector.tensor_mul(es[0][:, hw_:gw], es[1][:, hw_:gw], es[1][:, hw_:gw])
                pending.append((g, es))
                if len(pending) > WLAG:
                    emit_weighted(*pending.pop(0))
            for item in pending:
                emit_weighted(*item)

            # tail: drain each rq as its last matmul lands (q=4 first);
            # copies alternate DVE/ACT; bulk DMA after q=1, final q=0 alone.
            stg = ostage.tile([128, NQ * 4 * C], fp32)
            for q in range(NQ - 1, -1, -1):
                dst = stg[:, q * 4 * C : (q + 1) * 4 * C]
                if q % 2 == 0:
                    nc.vector.tensor_copy(dst, rq[q])
                else:
                    nc.scalar.activation(dst, rq[q], Copy)
                if q == 1:
                    nc.sync.dma_start(out=rout[:, 4 * C :], in_=stg[:, 4 * C :])
            nc.sync.dma_start(out=rout[:, 0 : 4 * C], in_=stg[:, 0 : 4 * C])

    nc.compile()
    return nc


def _jseq(c):
    seq = list(range(4 * c, 4 * c + 4))
    for d in range(NCORES):
        if d == c:
            continue
        if d > c:
            seq += [4 * d, 4 * d + 1, 4 * d + 2, 4 * d + 3]
        else:
            seq += [4 * d + 2, 4 * d + 3, 4 * d, 4 * d + 1]
    return seq


def _prep(source, target, source_label, target_logits):
    X = np.concatenate([np.asarray(source), np.asarray(target)], axis=0)
    X64 = X.astype(np.float64)
    sq = np.einsum("nd,nd->n", X64, X64)
    colsum = X64.sum(axis=0)
    sum_l2 = 2.0 * N * sq.sum() - 2.0 * (colsum @ colsum)
    bw = sum_l2 / (N * N - N) / (2.0 ** (NQ // 2))
    cq = np.array([1.0 / (bw * 2.0**q) for q in range(NQ)])  # [5]

    sl = np.asarray(source_label, np.float64)
    tl = np.asarray(target_logits, np.float64)
    ssum = sl.sum(0)
    s_norm = np.where(ssum > 0, sl / np.where(ssum > 0, ssum, 1.0), 0.0)
    tsum = tl.sum(0)
    t_norm = np.where(tsum > 0, tl / np.where(tsum > 0, tsum, 1.0), 0.0)
    s_pres = np.zeros(C)
    np.add.at(s_pres, sl.argmax(1), 1.0)
    t_pres = np.zeros(C)
    np.add.at(t_pres, tl.argmax(1), 1.0)
    common = ((s_pres > 0) & (t_pres > 0)).astype(np.float64)
    V = np.concatenate([s_norm * common, -t_norm * common], axis=0)  # [N, C]

    # fp8 X^T in [p, k, jcol] layout (global j order; per-core slot perm later)
    X8 = X.astype(F8NP)                                   # [N, D]
    xt8 = np.ascontiguousarray(
        X8.T.reshape(NKC, 128, N).transpose(1, 0, 2)      # [128, 8, N]
    )
    Vb = V.astype(BFNP)
    sqt = sq.reshape(NT, 128)
    return X, sq, cq, V, Vb, xt8, sqt


def _core_inputs(c, cq, Vb, xt8, sqt):
    seq = _jseq(c)
    # xtb: permute j-tiles into slot order
    xtb = np.ascontiguousarray(
        xt8.reshape(128, NKC, NT, 128)[:, :, seq, :].reshape(128, NKC, NT * 128)
    )
    Vt = Vb.astype(np.float64).reshape(NT, 128, C)[seq]   # [NT, 128, C]
    vt2 = (2.0 * Vt).transpose(1, 0, 2).reshape(128, NT * C)
    vt1 = Vt[:4].transpose(1, 0, 2).reshape(128, 4 * C)
    btab = np.ascontiguousarray(np.concatenate([vt2, vt1], axis=1)).astype(BFNP)
    ftab = np.zeros((128, NT + 2), np.float32)
    ftab[:, 0:NT] = (-cq[4] * sqt[seq]).T
    ftab[:, NT] = 2.0 * cq[4]
    return {"xtb": xtb, "ftab": ftab, "btab": btab}


def _postprocess(results, sq, cq, V):
    # loss = 1/12 sum_q sum_i alpha_q[i] * (sum_cls V[i,cls] R_q[cls,i])
    loss = 0.0
    for c in range(NCORES):
        # r[p, q, b, cls] = R_q[cls, i] at i = 512c + 128b + p
        r = np.asarray(results[c]["r_out"], np.float64).reshape(128, NQ, 4, C)
        gi = c * IPC + np.arange(IPC)
        Vc = V[gi].reshape(4, 128, C)                     # [b, p, cls]
        alpha = np.exp(-np.outer(cq, sq[gi])).reshape(NQ, 4, 128)
        loss += np.einsum("qbp,bpc,pqbc->", alpha, Vc, r)
    return loss / C


def _run(in_maps, trace=False, **kw):
    global _BUILT
    if _BUILT is None:
        _BUILT = _build_program()
    return run_bass_kernel_spmd(_BUILT, in_maps, list(range(NCORES)), trace=trace, **kw)


def kernel(source, target, source_label, target_logits, _trace=False, _ret_bkr=False):
    X, sq, cq, V, Vb, xt8, sqt = _prep(source, target, source_label, target_logits)
    in_maps = [_core_inputs(c, cq, Vb, xt8, sqt) for c in range(NCORES)]
    bkr = _run(in_maps, trace=_trace)
    loss = _postprocess(bkr.results, sq, cq, V)
    out = np.float32(loss)
    if _ret_bkr:
        return out, bkr
    return out


# revision 29
# speedup vs baseline: 3.7066x; 1.0179x over previous
"""LMMD (DSAN local MMD) loss on 8 Trainium2 NeuronCores — triangle + fp8 V2.

Math (reference):
    X = concat(source, target)                    # [N=4096, D=1024]
    l2[i,j] = max(|x_i|^2 + |x_j|^2 - 2 x_i.x_j, 0)
    bw      = sum(l2) / (N^2 - N) / 4
    K       = sum_q exp(-l2 / (bw * 2^q)),  q = 0..4
    loss    = sum_c v_c^T K v_c / 12,  V = [s_norm; -t_norm]  (rank-12 weights)

Design (vs full row-sharded baseline):
  * Triangle: K and the rank-12 weight matrix are symmetric, so each
    unordered tile-pair {u,v} of the 32x32 (j,i) 128-tile grid is computed
    once (weight 2 off-diagonal via vt2 = 2V, weight 1 diagonal via vt1 = V).
    A uniform edge-orientation makes the 8 per-core programs IDENTICAL:
    core c owns i-tiles [4c..4c+3]; its j-slot sequence is its own 4 tiles
    (suffix spans 512/384/256/128) then, per foreign core d, 4 slots with
    fixed i-column halves [0:256],[0:256],[256:512],[256:512], where the
    host permutes j-tiles (d>c: natural order, d<c: swapped halves) so that
    globally every pair is covered exactly once. 66 tile-jobs per core
    (vs 128 for the full row-block) — verified by brute force.
  * Gram in fp8e4m3 with MatmulPerfMode.DoubleRow: 4 matmuls per slot,
    each contracting 2 of the 8 128-deep k-chunks at 0.5 cycles/row.
    Host-side fp64 sq/bias tables keep l2 = sq_j + sq_i - 2G accurate;
    measured end-to-end rel err ~4e-3 (gate 2e-2).
  * es in bf16: e4 = exp(2 c4 G - c4 sq_j) on ACT (per-partition bias),
    then e3/e2/e1 by DVE squaring (2x_1p), e0 split Pool/DVE — batched
    across a group's 4 slots in one wide SBUF tile.
  * Weighted reduce FLIPPED: the es block (128j x 128i) is the STATIONARY
    PE operand and the 12-class vt block is the moving tensor, so each
    accumulation matmul streams 12 rows instead of 128 (and fills the PE
    array 128-wide instead of 12-wide — also the right choice on real hw).
    R_q[i, cls] accumulates in PSUM per own 128-col block; host applies
    alpha_q[i] = exp(-c_q sq_i) and the final V-contraction.
  * DMA: xtb staged host-side as [128, 8k, 4096] fp8 so one dma_start
    moves a whole 512-column j-group (the shared HWDGE device costs
    625ns per dma_start — count matters).
"""

import numpy as np
import ml_dtypes

import concourse.bass as bass
from concourse import bacc
import concourse.mybir as mybir
import concourse.tile as tile
from concourse.bass_utils import run_bass_kernel_spmd

B = 2048
D = 1024
C = 12
NCORES = 8
N = 2 * B                 # 4096 total samples
IPC = N // NCORES         # 512 own columns (i) per core
NT = N // 128             # 32 j-tiles
NKC = D // 128            # 8 contraction chunks
NKP = NKC // 2            # 4 DoubleRow chunk-pairs
NQ = 5                    # kernels in the RBF mixture
NGROUPS = 8               # slot groups: own + 7 foreign cores
OWN_G = 0                 # own group first (cheapest start: no weight DMA)
WLAG = 3                  # groups of lag between es production and weighted use

# (block_off, block_end) in 128-col i-blocks, per slot-in-group
OWN_SPANS = [(0, 4), (1, 4), (2, 4), (3, 4)]
FOREIGN_PAT = [(0, 2), (0, 2), (2, 4), (2, 4)]

F8NP = ml_dtypes.float8_e4m3
BFNP = ml_dtypes.bfloat16

_BUILT = None             # program is input-independent


def _slot_geom(g, sl):
    if g == OWN_G:
        off, end = OWN_SPANS[sl]
    else:
        off, end = FOREIGN_PAT[sl]
    return off, end


def _ebase(g, sl):
    # column base of slot sl inside the group's batched e tiles
    bases = [0]
    for s in range(1, 4):
        o, e = _slot_geom(g, s - 1)
        bases.append(bases[-1] + (e - o) * 128)
    return bases[sl]


def _build_program():
    fp32 = mybir.dt.float32
    bf16 = mybir.dt.bfloat16
    f8 = mybir.dt.float8e4
    Exp = mybir.ActivationFunctionType.Exp
    Copy = mybir.ActivationFunctionType.Copy
    DR = mybir.MatmulPerfMode.DoubleRow

    nc = bacc.Bacc()
    # host-pretransposed: xtb[p, k, s*128+j] = X[jseq[s]*128+j, k*128+p]
    xtb = nc.declare_dram_parameter("xtb", [128, NKC, NT * 128], f8, isOutput=False)
    # ftab = qsq[NT] | scl[2];  btab = vt2[NT*C] | vt1[4*C]
    ftab = nc.declare_dram_parameter("ftab", [128, NT + 2], fp32, isOutput=False)
    btab = nc.declare_dram_parameter("btab", [128, NT * C + 4 * C], bf16, isOutput=False)
    # flipped weighted layout: rows = i within own 128-block, cols = (q, block, cls)
    rout = nc.declare_dram_parameter("r_out", [128, NQ * 4 * C], fp32, isOutput=True)

    with tile.TileContext(nc) as tc:
        with (
            tc.tile_pool(name="singles", bufs=1) as singles,
            tc.tile_pool(name="wpool", bufs=3) as wpool,
            tc.tile_pool(name="epool", bufs=5) as epool,
            tc.tile_pool(name="ostage", bufs=1) as ostage,
            tc.tile_pool(name="gpsum", bufs=3, space="PSUM") as gpsum,
            tc.tile_pool(name="rqpsum", bufs=1, space="PSUM") as rqpsum,
        ):
            # own i-columns = slots 0..3 of xtb, first on the serialized DMA
            # path (gram slot 0 starts after the first half); small tables
            # follow in consumer order (exp bias/scale before vt).
            own_sb = singles.tile([128, NKC, IPC], f8)
            nc.sync.dma_start(out=own_sb[:, 0 : NKC // 2, :], in_=xtb[:, 0 : NKC // 2, 0:IPC])
            nc.sync.dma_start(out=own_sb[:, NKC // 2 : NKC, :], in_=xtb[:, NKC // 2 : NKC, 0:IPC])
            ftab_sb = singles.tile([128, NT + 2], fp32)
            nc.sync.dma_start(out=ftab_sb, in_=ftab[:])
            btab_sb = singles.tile([128, NT * C + 4 * C], bf16)
            nc.sync.dma_start(out=btab_sb, in_=btab[:])
            # Stage small tiles through DVE so consumers wait on one
            # semaphore instead of the DMA queue fan-out (walrus caps the
            # per-instruction sync-wait count).
            qsq_s = singles.tile([128, NT], fp32)
            nc.vector.tensor_copy(qsq_s, ftab_sb[:, 0:NT])
            scl_s = singles.tile([128, 2], fp32)
            nc.vector.tensor_copy(scl_s, ftab_sb[:, NT : NT + 2])
            vt2_s = singles.tile([128, NT * C], bf16)
            nc.vector.tensor_copy(vt2_s, btab_sb[:, 0 : NT * C])
            vt1_s = singles.tile([128, 4 * C], bf16)
            nc.vector.tensor_copy(vt1_s, btab_sb[:, NT * C : NT * C + 4 * C])
            # Dummy ACT op: loads the Exp table early and absorbs the DVE
            # wait so loop Exp ops only ever need the PE wait.
            warm = singles.tile([128, 2], fp32)
            nc.scalar.activation(warm, scl_s, Exp)

            # rq[q][i, b*C+cls] accumulates R_q over j for own block b
            rq = [rqpsum.tile([128, 4 * C], fp32, tag=f"rq{q}", name=f"rq{q}") for q in range(NQ)]

            def emit_weighted(g, es):
                # Flipped orientation: es block stationary, vt moving.
                # q-major, q=4 first so PE chases the DVE squaring chain.
                # PSUM zero-region semantics: exactly one start (first matmul
                # into the bank) and one stop (last) per rq tile.
                for q in range(NQ - 1, -1, -1):
                    for sl in range(4):
                        slot = 4 * g + sl
                        off, end = _slot_geom(g, sl)
                        eb = _ebase(g, sl)
                        for b in range(off, end):
                            if g == OWN_G and b == sl:
                                vtb = vt1_s[:, sl * C : (sl + 1) * C]
                            else:
                                vtb = vt2_s[:, slot * C : (slot + 1) * C]
                            col = eb + (b - off) * 128
                            nc.tensor.matmul(
                                rq[q][:, b * C : (b + 1) * C],
                                lhsT=es[q][:, col : col + 128],
                                rhs=vtb,
                                start=(g == 0 and sl == 0 and b == 0),
                                stop=(g == NGROUPS - 1 and sl == 3 and b == 3),
                            )

            pending = []
            for g in range(NGROUPS):
                if g == OWN_G:
                    wsrc = own_sb
                else:
                    wg = wpool.tile([128, NKC, 512], f8, tag="wg", name=f"w{g}")
                    src0 = g * 512
                    nc.sync.dma_start(out=wg, in_=xtb[:, :, src0 : src0 + 512])
                    wsrc = wg
                gw = 1280 if g == OWN_G else 1024  # total e-batch width
                es = {q: epool.tile([128, 1280], bf16, tag=f"e{q}", name=f"e{q}g{g}") for q in range(NQ)}
                for sl in (range(3, -1, -1) if g == OWN_G else range(4)):
                    slot = 4 * g + sl
                    off, end = _slot_geom(g, sl)
                    span = (end - off) * 128
                    eb = _ebase(g, sl)
                    gt = gpsum.tile([128, IPC], fp32, tag="g", name=f"g{slot}")
                    for m in range(NKP):
                        nc.tensor.matmul(
                            gt[:, 0:span],
                            lhsT=wsrc[:, 2 * m : 2 * m + 2, sl * 128 : (sl + 1) * 128],
                            rhs=own_sb[:, 2 * m : 2 * m + 2, off * 128 : end * 128],
                            start=(m == 0),
                            stop=(m == NKP - 1),
                            perf_mode=DR,
                        )
                    nc.scalar.activation(
                        es[4][:, eb : eb + span],
                        gt[:, 0:span],
                        Exp,
                        bias=qsq_s[:, slot : slot + 1],
                        scale=scl_s[:, 0:1],
                    )
                # squaring chain: e3/e2/e1 on DVE (2x_1p), e0 split between
                # the idle Pool engine and DVE
                nc.vector.tensor_mul(es[3][:, 0:gw], es[4][:, 0:gw], es[4][:, 0:gw])
                nc.vector.tensor_mul(es[2][:, 0:gw], es[3][:, 0:gw], es[3][:, 0:gw])
                nc.vector.tensor_mul(es[1][:, 0:gw], es[2][:, 0:gw], es[2][:, 0:gw])
                hw_ = 21 * gw // 32
                nc.gpsimd.tensor_mul(es[0][:, 0:hw_], es[1][:, 0:hw_], es[1][:, 0:hw_])
                nc.vector.tensor_mul(es[0][:, hw_:gw], es[1][:, hw_:gw], es[1][:, hw_:gw])
                pending.append((g, es))
                if len(pending) > WLAG:
                    emit_weighted(*pending.pop(0))
            for item in pending:
                emit_weighted(*item)

            # tail: drain each rq as its last matmul lands (q=4 first);
            # copies alternate DVE/ACT; bulk DMA after q=1, final q=0 alone.
            stg = ostage.tile([128, NQ * 4 * C], fp32)
            for q in range(NQ - 1, -1, -1):
                dst = stg[:, q * 4 * C : (q + 1) * 4 * C]
                if q % 2 == 0:
                    nc.vector.tensor_copy(dst, rq[q])
                else:
                    nc.scalar.activation(dst, rq[q], Copy)
                if q == 1:
                    nc.sync.dma_start(out=rout[:, 4 * C :], in_=stg[:, 4 * C :])
            nc.sync.dma_start(out=rout[:, 0 : 4 * C], in_=stg[:, 0 : 4 * C])

    nc.compile()
    return nc


def _jseq(c):
    seq = list(range(4 * c, 4 * c + 4))
    for d in range(NCORES):
        if d == c:
            continue
        if d > c:
            seq += [4 * d, 4 * d + 1, 4 * d + 2, 4 * d + 3]
        else:
            seq += [4 * d + 2, 4 * d + 3, 4 * d, 4 * d + 1]
    return seq


def _prep(source, target, source_label, target_logits):
    X = np.concatenate([np.asarray(source), np.asarray(target)], axis=0)
    X64 = X.astype(np.float64)
    sq = np.einsum("nd,nd->n", X64, X64)
    colsum = X64.sum(axis=0)
    sum_l2 = 2.0 * N * sq.sum() - 2.0 * (colsum @ colsum)
    bw = sum_l2 / (N * N - N) / (2.0 ** (NQ // 2))
    cq = np.array([1.0 / (bw * 2.0**q) for q in range(NQ)])  # [5]

    sl = np.asarray(source_label, np.float64)
    tl = np.asarray(target_logits, np.float64)
    ssum = sl.sum(0)
    s_norm = np.where(ssum > 0, sl / np.where(ssum > 0, ssum, 1.0), 0.0)
    tsum = tl.sum(0)
    t_norm = np.where(tsum > 0, tl / np.where(tsum > 0, tsum, 1.0), 0.0)
    s_pres = np.zeros(C)
    np.add.at(s_pres, sl.argmax(1), 1.0)
    t_pres = np.zeros(C)
    np.add.at(t_pres, tl.argmax(1), 1.0)
    common = ((s_pres > 0) & (t_pres > 0)).astype(np.float64)
    V = np.concatenate([s_norm * common, -t_norm * common], axis=0)  # [N, C]

    # fp8 X^T in [p, k, jcol] layout (global j order; per-core slot perm later)
    X8 = X.astype(F8NP)                                   # [N, D]
    xt8 = np.ascontiguousarray(
        X8.T.reshape(NKC, 128, N).transpose(1, 0, 2)      # [128, 8, N]
    )
    Vb = V.astype(BFNP)
    sqt = sq.reshape(NT, 128)
    return X, sq, cq, V, Vb, xt8, sqt


def _core_inputs(c, cq, Vb, xt8, sqt):
    seq = _jseq(c)
    # xtb: permute j-tiles into slot order
    xtb = np.ascontiguousarray(
        xt8.reshape(128, NKC, NT, 128)[:, :, seq, :].reshape(128, NKC, NT * 128)
    )
    Vt = Vb.astype(np.float64).reshape(NT, 128, C)[seq]   # [NT, 128, C]
    vt2 = (2.0 * Vt).transpose(1, 0, 2).reshape(128, NT * C)
    vt1 = Vt[:4].transpose(1, 0, 2).reshape(128, 4 * C)
    btab = np.ascontiguousarray(np.concatenate([vt2, vt1], axis=1)).astype(BFNP)
    ftab = np.zeros((128, NT + 2), np.float32)
    ftab[:, 0:NT] = (-cq[4] * sqt[seq]).T
    ftab[:, NT] = 2.0 * cq[4]
    return {"xtb": xtb, "ftab": ftab, "btab": btab}


def _postprocess(results, sq, cq, V):
    # loss = 1/12 sum_q sum_i alpha_q[i] * (sum_cls V[i,cls] R_q[cls,i])
    loss = 0.0
    for c in range(NCORES):
        # r[p, q, b, cls] = R_q[cls, i] at i = 512c + 128b + p
        r = np.asarray(results[c]["r_out"], np.float64).reshape(128, NQ, 4, C)
        gi = c * IPC + np.arange(IPC)
        Vc = V[gi].reshape(4, 128, C)                     # [b, p, cls]
        alpha = np.exp(-np.outer(cq, sq[gi])).reshape(NQ, 4, 128)
        loss += np.einsum("qbp,bpc,pqbc->", alpha, Vc, r)
    return loss / C


def _run(in_maps, trace=False, **kw):
    global _BUILT
    if _BUILT is None:
        _BUILT = _build_program()
    return run_bass_kernel_spmd(_BUILT, in_maps, list(range(NCORES)), trace=trace, **kw)


def kernel(source, target, source_label, target_logits, _trace=False, _ret_bkr=False):
    X, sq, cq, V, Vb, xt8, sqt = _prep(source, target, source_label, target_logits)
    in_maps = [_core_inputs(c, cq, Vb, xt8, sqt) for c in range(NCORES)]
    try:
        bkr = _run(in_maps, trace=_trace)
    except Exception:
        # transient device wedge (NRT_EXEC_UNIT_UNRECOVERABLE) — retry once
        bkr = _run(in_maps, trace=_trace)
    loss = _postprocess(bkr.results, sq, cq, V)
    out = np.float32(loss)
    if _ret_bkr:
        return out, bkr
    return out


# revision 35
# speedup vs baseline: 3.8540x; 1.0398x over previous
"""LMMD (DSAN local MMD) loss on 8 Trainium2 NeuronCores — triangle + fp8 V2.

Math (reference):
    X = concat(source, target)                    # [N=4096, D=1024]
    l2[i,j] = max(|x_i|^2 + |x_j|^2 - 2 x_i.x_j, 0)
    bw      = sum(l2) / (N^2 - N) / 4
    K       = sum_q exp(-l2 / (bw * 2^q)),  q = 0..4
    loss    = sum_c v_c^T K v_c / 12,  V = [s_norm; -t_norm]  (rank-12 weights)

Design (vs full row-sharded baseline):
  * Triangle: K and the rank-12 weight matrix are symmetric, so each
    unordered tile-pair {u,v} of the 32x32 (j,i) 128-tile grid is computed
    once (weight 2 off-diagonal via vt2 = 2V, weight 1 diagonal via vt1 = V).
    A uniform edge-orientation makes the 8 per-core programs IDENTICAL:
    core c owns i-tiles [4c..4c+3]; its j-slot sequence is its own 4 tiles
    (suffix spans 512/384/256/128) then, per foreign core d, 4 slots with
    fixed i-column halves [0:256],[0:256],[256:512],[256:512], where the
    host permutes j-tiles (d>c: natural order, d<c: swapped halves) so that
    globally every pair is covered exactly once. 66 tile-jobs per core
    (vs 128 for the full row-block) — verified by brute force.
  * Gram in fp8e4m3 with MatmulPerfMode.DoubleRow: 4 matmuls per slot,
    each contracting 2 of the 8 128-deep k-chunks at 0.5 cycles/row.
    Host-side fp64 sq/bias tables keep l2 = sq_j + sq_i - 2G accurate;
    measured end-to-end rel err ~4e-3 (gate 2e-2).
  * es in bf16: e4 = exp(2 c4 G - c4 sq_j) on ACT (per-partition bias),
    then e3/e2/e1 by DVE squaring (2x_1p), e0 split Pool/DVE — batched
    across a group's 4 slots in one wide SBUF tile.
  * Weighted reduce FLIPPED: the es block (128j x 128i) is the STATIONARY
    PE operand and the 12-class vt block is the moving tensor, so each
    accumulation matmul streams 12 rows instead of 128 (and fills the PE
    array 128-wide instead of 12-wide — also the right choice on real hw).
    R_q[i, cls] accumulates in PSUM per own 128-col block; host applies
    alpha_q[i] = exp(-c_q sq_i) and the final V-contraction.
  * DMA: xtb staged host-side as [128, 8k, 4096] fp8 so one dma_start
    moves a whole 512-column j-group (the shared HWDGE device costs
    625ns per dma_start — count matters).
"""

import numpy as np
import ml_dtypes

import concourse.bass as bass
from concourse import bacc
import concourse.mybir as mybir
import concourse.tile as tile
from concourse.bass_utils import run_bass_kernel_spmd

B = 2048
D = 1024
C = 12
NCORES = 8
N = 2 * B                 # 4096 total samples
IPC = N // NCORES         # 512 own columns (i) per core
NT = N // 128             # 32 j-tiles
NKC = D // 128            # 8 contraction chunks
NKP = NKC // 2            # 4 DoubleRow chunk-pairs
NQ = 5                    # kernels in the RBF mixture
NGROUPS = 8               # slot groups: own + 7 foreign cores
OWN_G = 0                 # own group first (cheapest start: no weight DMA)
WLAG = 3                  # groups of lag between es production and weighted use

# (block_off, block_end) in 128-col i-blocks, per slot-in-group
OWN_SPANS = [(0, 4), (1, 4), (2, 4), (3, 4)]
FOREIGN_PAT = [(0, 2), (0, 2), (2, 4), (2, 4)]

F8NP = ml_dtypes.float8_e4m3
BFNP = ml_dtypes.bfloat16

_BUILT = None             # program is input-independent


def _slot_geom(g, sl):
    if g == OWN_G:
        off, end = OWN_SPANS[sl]
    else:
        off, end = FOREIGN_PAT[sl]
    return off, end


def _ebase(g, sl):
    # column base of slot sl inside the group's batched e tiles
    bases = [0]
    for s in range(1, 4):
        o, e = _slot_geom(g, s - 1)
        bases.append(bases[-1] + (e - o) * 128)
    return bases[sl]


def _build_program():
    fp32 = mybir.dt.float32
    bf16 = mybir.dt.bfloat16
    f8 = mybir.dt.float8e4
    Exp = mybir.ActivationFunctionType.Exp
    Copy = mybir.ActivationFunctionType.Copy
    DR = mybir.MatmulPerfMode.DoubleRow

    nc = bacc.Bacc()
    # host-pretransposed: xtb[p, k, s*128+j] = X[jseq[s]*128+j, k*128+p]
    xtb = nc.declare_dram_parameter("xtb", [128, NKC, NT * 128], f8, isOutput=False)
    # ftab = qsq[NT] | scl[2];  btab = vt2[NT*C] | vt1[4*C]
    ftab = nc.declare_dram_parameter("ftab", [128, NT + 2], fp32, isOutput=False)
    btab = nc.declare_dram_parameter("btab", [128, NT * C + 4 * C], bf16, isOutput=False)
    # flipped weighted layout: rows = i within own 128-block, cols = (q, block, cls)
    rout = nc.declare_dram_parameter("r_out", [128, NQ * 4 * C], fp32, isOutput=True)

    with tile.TileContext(nc) as tc:
        with (
            tc.tile_pool(name="singles", bufs=1) as singles,
            tc.tile_pool(name="wpool", bufs=3) as wpool,
            tc.tile_pool(name="epool", bufs=5) as epool,
            tc.tile_pool(name="ostage", bufs=1) as ostage,
            tc.tile_pool(name="gpsum", bufs=3, space="PSUM") as gpsum,
            tc.tile_pool(name="rqpsum", bufs=1, space="PSUM") as rqpsum,
        ):
            # own i-columns = slots 0..3 of xtb, first on the serialized DMA
            # path (gram slot 0 starts after the first half); small tables
            # follow in consumer order (exp bias/scale before vt).
            own_sb = singles.tile([128, NKC, IPC], f8)
            nc.sync.dma_start(out=own_sb[:, 0 : NKC // 2, :], in_=xtb[:, 0 : NKC // 2, 0:IPC])
            nc.sync.dma_start(out=own_sb[:, NKC // 2 : NKC, :], in_=xtb[:, NKC // 2 : NKC, 0:IPC])
            ftab_sb = singles.tile([128, NT + 2], fp32)
            nc.sync.dma_start(out=ftab_sb, in_=ftab[:])
            btab_sb = singles.tile([128, NT * C + 4 * C], bf16)
            # Stage small tiles through DVE so consumers wait on one
            # semaphore instead of the DMA queue fan-out (walrus caps the
            # per-instruction sync-wait count).
            qsq_s = singles.tile([128, NT], fp32)
            nc.vector.tensor_copy(qsq_s, ftab_sb[:, 0:NT])
            scl_s = singles.tile([128, 2], fp32)
            nc.vector.tensor_copy(scl_s, ftab_sb[:, NT : NT + 2])
            vt2_s = singles.tile([128, NT * C], bf16)
            vt1_s = singles.tile([128, 4 * C], bf16)
            # Dummy ACT op: loads the Exp table early and absorbs the DVE
            # wait so loop Exp ops only ever need the PE wait.
            warm = singles.tile([128, 2], fp32)
            nc.scalar.activation(warm, scl_s, Exp)

            # rq[q][i, b*C+cls] accumulates R_q over j for own block b
            rq = [rqpsum.tile([128, 4 * C], fp32, tag=f"rq{q}", name=f"rq{q}") for q in range(NQ)]

            def emit_weighted(g, es):
                # Flipped orientation: es block stationary, vt moving.
                # q-major, q=4 first so PE chases the DVE squaring chain.
                # PSUM zero-region semantics: exactly one start (first matmul
                # into the bank) and one stop (last) per rq tile.
                for q in range(NQ - 1, -1, -1):
                    for sl in range(4):
                        slot = 4 * g + sl
                        off, end = _slot_geom(g, sl)
                        eb = _ebase(g, sl)
                        for b in range(off, end):
                            if g == OWN_G and b == sl:
                                vtb = vt1_s[:, sl * C : (sl + 1) * C]
                            else:
                                vtb = vt2_s[:, slot * C : (slot + 1) * C]
                            col = eb + (b - off) * 128
                            nc.tensor.matmul(
                                rq[q][:, b * C : (b + 1) * C],
                                lhsT=es[q][:, col : col + 128],
                                rhs=vtb,
                                start=(g == 0 and sl == 0 and b == 0),
                                stop=(g == NGROUPS - 1 and sl == 3 and b == 3),
                            )

            pending = []
            for g in range(NGROUPS):
                if g == OWN_G:
                    wsrc = own_sb
                else:
                    wg = wpool.tile([128, NKC, 512], f8, tag="wg", name=f"w{g}")
                    src0 = g * 512
                    nc.sync.dma_start(out=wg, in_=xtb[:, :, src0 : src0 + 512])
                    if g == 1:
                        # vt tables land late (first consumer is weighted(0)
                        # at ~6us) so wg1/wg2 win the serialized DMA path
                        nc.sync.dma_start(out=btab_sb, in_=btab[:])
                        nc.vector.tensor_copy(vt2_s, btab_sb[:, 0 : NT * C])
                        nc.vector.tensor_copy(vt1_s, btab_sb[:, NT * C : NT * C + 4 * C])
                    wsrc = wg
                gw = 1280 if g == OWN_G else 1024  # total e-batch width
                es = {q: epool.tile([128, 1280], bf16, tag=f"e{q}", name=f"e{q}g{g}") for q in range(NQ)}
                for sl in (range(3, -1, -1) if g == OWN_G else range(4)):
                    slot = 4 * g + sl
                    off, end = _slot_geom(g, sl)
                    span = (end - off) * 128
                    eb = _ebase(g, sl)
                    gt = gpsum.tile([128, IPC], fp32, tag="g", name=f"g{slot}")
                    for m in range(NKP):
                        nc.tensor.matmul(
                            gt[:, 0:span],
                            lhsT=wsrc[:, 2 * m : 2 * m + 2, sl * 128 : (sl + 1) * 128],
                            rhs=own_sb[:, 2 * m : 2 * m + 2, off * 128 : end * 128],
                            start=(m == 0),
                            stop=(m == NKP - 1),
                            perf_mode=DR,
                        )
                    nc.scalar.activation(
                        es[4][:, eb : eb + span],
                        gt[:, 0:span],
                        Exp,
                        bias=qsq_s[:, slot : slot + 1],
                        scale=scl_s[:, 0:1],
                    )
                # squaring chain: e3/e2/e1 on DVE (2x_1p), e0 split between
                # the idle Pool engine and DVE
                nc.vector.tensor_mul(es[3][:, 0:gw], es[4][:, 0:gw], es[4][:, 0:gw])
                nc.vector.tensor_mul(es[2][:, 0:gw], es[3][:, 0:gw], es[3][:, 0:gw])
                nc.vector.tensor_mul(es[1][:, 0:gw], es[2][:, 0:gw], es[2][:, 0:gw])
                hw_ = 0 if g == NGROUPS - 1 else 13 * gw // 16
                if hw_:
                    nc.gpsimd.tensor_mul(es[0][:, 0:hw_], es[1][:, 0:hw_], es[1][:, 0:hw_])
                nc.vector.tensor_mul(es[0][:, hw_:gw], es[1][:, hw_:gw], es[1][:, hw_:gw])
                pending.append((g, es))
                if len(pending) > WLAG:
                    emit_weighted(*pending.pop(0))
            for item in pending:
                emit_weighted(*item)

            # tail: drain each rq as its last matmul lands (q=4 first);
            # copies alternate DVE/ACT; bulk DMA after q=1, final q=0 alone.
            stg = ostage.tile([128, NQ * 4 * C], fp32)
            for q in range(NQ - 1, -1, -1):
                dst = stg[:, q * 4 * C : (q + 1) * 4 * C]
                if q % 2 == 0:
                    nc.vector.tensor_copy(dst, rq[q])
                else:
                    nc.scalar.activation(dst, rq[q], Copy)
                if q == 1:
                    nc.sync.dma_start(out=rout[:, 4 * C :], in_=stg[:, 4 * C :])
            nc.sync.dma_start(out=rout[:, 0 : 4 * C], in_=stg[:, 0 : 4 * C])

    nc.compile()
    return nc


def _jseq(c):
    seq = list(range(4 * c, 4 * c + 4))
    for d in range(NCORES):
        if d == c:
            continue
        if d > c:
            seq += [4 * d, 4 * d + 1, 4 * d + 2, 4 * d + 3]
        else:
            seq += [4 * d + 2, 4 * d + 3, 4 * d, 4 * d + 1]
    return seq


def _prep(source, target, source_label, target_logits):
    X = np.concatenate([np.asarray(source), np.asarray(target)], axis=0)
    X64 = X.astype(np.float64)
    sq = np.einsum("nd,nd->n", X64, X64)
    colsum = X64.sum(axis=0)
    sum_l2 = 2.0 * N * sq.sum() - 2.0 * (colsum @ colsum)
    bw = sum_l2 / (N * N - N) / (2.0 ** (NQ // 2))
    cq = np.array([1.0 / (bw * 2.0**q) for q in range(NQ)])  # [5]

    sl = np.asarray(source_label, np.float64)
    tl = np.asarray(target_logits, np.float64)
    ssum = sl.sum(0)
    s_norm = np.where(ssum > 0, sl / np.where(ssum > 0, ssum, 1.0), 0.0)
    tsum = tl.sum(0)
    t_norm = np.where(tsum > 0, tl / np.where(tsum > 0, tsum, 1.0), 0.0)
    s_pres = np.zeros(C)
    np.add.at(s_pres, sl.argmax(1), 1.0)
    t_pres = np.zeros(C)
    np.add.at(t_pres, tl.argmax(1), 1.0)
    common = ((s_pres > 0) & (t_pres > 0)).astype(np.float64)
    V = np.concatenate([s_norm * common, -t_norm * common], axis=0)  # [N, C]

    # fp8 X^T in [p, k, jcol] layout (global j order; per-core slot perm later)
    X8 = X.astype(F8NP)                                   # [N, D]
    xt8 = np.ascontiguousarray(
        X8.T.reshape(NKC, 128, N).transpose(1, 0, 2)      # [128, 8, N]
    )
    Vb = V.astype(BFNP)
    sqt = sq.reshape(NT, 128)
    return X, sq, cq, V, Vb, xt8, sqt


def _core_inputs(c, cq, Vb, xt8, sqt):
    seq = _jseq(c)
    # xtb: permute j-tiles into slot order
    xtb = np.ascontiguousarray(
        xt8.reshape(128, NKC, NT, 128)[:, :, seq, :].reshape(128, NKC, NT * 128)
    )
    Vt = Vb.astype(np.float64).reshape(NT, 128, C)[seq]   # [NT, 128, C]
    vt2 = (2.0 * Vt).transpose(1, 0, 2).reshape(128, NT * C)
    vt1 = Vt[:4].transpose(1, 0, 2).reshape(128, 4 * C)
    btab = np.ascontiguousarray(np.concatenate([vt2, vt1], axis=1)).astype(BFNP)
    ftab = np.zeros((128, NT + 2), np.float32)
    ftab[:, 0:NT] = (-cq[4] * sqt[seq]).T
    ftab[:, NT] = 2.0 * cq[4]
    return {"xtb": xtb, "ftab": ftab, "btab": btab}


def _postprocess(results, sq, cq, V):
    # loss = 1/12 sum_q sum_i alpha_q[i] * (sum_cls V[i,cls] R_q[cls,i])
    loss = 0.0
    for c in range(NCORES):
        # r[p, q, b, cls] = R_q[cls, i] at i = 512c + 128b + p
        r = np.asarray(results[c]["r_out"], np.float64).reshape(128, NQ, 4, C)
        gi = c * IPC + np.arange(IPC)
        Vc = V[gi].reshape(4, 128, C)                     # [b, p, cls]
        alpha = np.exp(-np.outer(cq, sq[gi])).reshape(NQ, 4, 128)
        loss += np.einsum("qbp,bpc,pqbc->", alpha, Vc, r)
    return loss / C


def _run(in_maps, trace=False, **kw):
    global _BUILT
    if _BUILT is None:
        _BUILT = _build_program()
    return run_bass_kernel_spmd(_BUILT, in_maps, list(range(NCORES)), trace=trace, **kw)


def kernel(source, target, source_label, target_logits, _trace=False, _ret_bkr=False):
    X, sq, cq, V, Vb, xt8, sqt = _prep(source, target, source_label, target_logits)
    in_maps = [_core_inputs(c, cq, Vb, xt8, sqt) for c in range(NCORES)]
    try:
        bkr = _run(in_maps, trace=_trace)
    except Exception:
        # transient device wedge (NRT_EXEC_UNIT_UNRECOVERABLE) — retry once
        bkr = _run(in_maps, trace=_trace)
    loss = _postprocess(bkr.results, sq, cq, V)
    out = np.float32(loss)
    if _ret_bkr:
        return out, bkr
    return out


# revision 36
# speedup vs baseline: 3.8597x; 1.0015x over previous
"""LMMD (DSAN local MMD) loss on 8 Trainium2 NeuronCores — triangle + fp8 V2.

Math (reference):
    X = concat(source, target)                    # [N=4096, D=1024]
    l2[i,j] = max(|x_i|^2 + |x_j|^2 - 2 x_i.x_j, 0)
    bw      = sum(l2) / (N^2 - N) / 4
    K       = sum_q exp(-l2 / (bw * 2^q)),  q = 0..4
    loss    = sum_c v_c^T K v_c / 12,  V = [s_norm; -t_norm]  (rank-12 weights)

Design (vs full row-sharded baseline):
  * Triangle: K and the rank-12 weight matrix are symmetric, so each
    unordered tile-pair {u,v} of the 32x32 (j,i) 128-tile grid is computed
    once (weight 2 off-diagonal via vt2 = 2V, weight 1 diagonal via vt1 = V).
    A uniform edge-orientation makes the 8 per-core programs IDENTICAL:
    core c owns i-tiles [4c..4c+3]; its j-slot sequence is its own 4 tiles
    (suffix spans 512/384/256/128) then, per foreign core d, 4 slots with
    fixed i-column halves [0:256],[0:256],[256:512],[256:512], where the
    host permutes j-tiles (d>c: natural order, d<c: swapped halves) so that
    globally every pair is covered exactly once. 66 tile-jobs per core
    (vs 128 for the full row-block) — verified by brute force.
  * Gram in fp8e4m3 with MatmulPerfMode.DoubleRow: 4 matmuls per slot,
    each contracting 2 of the 8 128-deep k-chunks at 0.5 cycles/row.
    Host-side fp64 sq/bias tables keep l2 = sq_j + sq_i - 2G accurate;
    measured end-to-end rel err ~4e-3 (gate 2e-2).
  * es in bf16: e4 = exp(2 c4 G - c4 sq_j) on ACT (per-partition bias),
    then e3/e2/e1 by DVE squaring (2x_1p), e0 split Pool/DVE — batched
    across a group's 4 slots in one wide SBUF tile.
  * Weighted reduce FLIPPED: the es block (128j x 128i) is the STATIONARY
    PE operand and the 12-class vt block is the moving tensor, so each
    accumulation matmul streams 12 rows instead of 128 (and fills the PE
    array 128-wide instead of 12-wide — also the right choice on real hw).
    R_q[i, cls] accumulates in PSUM per own 128-col block; host applies
    alpha_q[i] = exp(-c_q sq_i) and the final V-contraction.
  * DMA: xtb staged host-side as [128, 8k, 4096] fp8 so one dma_start
    moves a whole 512-column j-group (the shared HWDGE device costs
    625ns per dma_start — count matters).
"""

import numpy as np
import ml_dtypes

import concourse.bass as bass
from concourse import bacc
import concourse.mybir as mybir
import concourse.tile as tile
from concourse.bass_utils import run_bass_kernel_spmd

B = 2048
D = 1024
C = 12
NCORES = 8
N = 2 * B                 # 4096 total samples
IPC = N // NCORES         # 512 own columns (i) per core
NT = N // 128             # 32 j-tiles
NKC = D // 128            # 8 contraction chunks
NKP = NKC // 2            # 4 DoubleRow chunk-pairs
NQ = 5                    # kernels in the RBF mixture
NGROUPS = 8               # slot groups: own + 7 foreign cores
OWN_G = 0                 # own group first (cheapest start: no weight DMA)
WLAG = 3                  # groups of lag between es production and weighted use

# (block_off, block_end) in 128-col i-blocks, per slot-in-group
OWN_SPANS = [(0, 4), (1, 4), (2, 4), (3, 4)]
FOREIGN_PAT = [(0, 2), (0, 2), (2, 4), (2, 4)]

F8NP = ml_dtypes.float8_e4m3
BFNP = ml_dtypes.bfloat16

_BUILT = None             # program is input-independent


def _slot_geom(g, sl):
    if g == OWN_G:
        off, end = OWN_SPANS[sl]
    else:
        off, end = FOREIGN_PAT[sl]
    return off, end


def _ebase(g, sl):
    # column base of slot sl inside the group's batched e tiles
    bases = [0]
    for s in range(1, 4):
        o, e = _slot_geom(g, s - 1)
        bases.append(bases[-1] + (e - o) * 128)
    return bases[sl]


def _build_program():
    fp32 = mybir.dt.float32
    bf16 = mybir.dt.bfloat16
    f8 = mybir.dt.float8e4
    Exp = mybir.ActivationFunctionType.Exp
    Copy = mybir.ActivationFunctionType.Copy
    DR = mybir.MatmulPerfMode.DoubleRow

    nc = bacc.Bacc()
    # host-pretransposed: xtb[p, k, s*128+j] = X[jseq[s]*128+j, k*128+p]
    xtb = nc.declare_dram_parameter("xtb", [128, NKC, NT * 128], f8, isOutput=False)
    # ftab = qsq[NT] | scl[2];  btab = vt2[NT*C] | vt1[4*C]
    ftab = nc.declare_dram_parameter("ftab", [128, NT + 2], fp32, isOutput=False)
    btab = nc.declare_dram_parameter("btab", [128, NT * C + 4 * C], bf16, isOutput=False)
    # flipped weighted layout: rows = i within own 128-block, cols = (q, block, cls)
    rout = nc.declare_dram_parameter("r_out", [128, NQ * 4 * C], fp32, isOutput=True)

    with tile.TileContext(nc) as tc:
        with (
            tc.tile_pool(name="singles", bufs=1) as singles,
            tc.tile_pool(name="wpool", bufs=3) as wpool,
            tc.tile_pool(name="epool", bufs=5) as epool,
            tc.tile_pool(name="ostage", bufs=1) as ostage,
            tc.tile_pool(name="gpsum", bufs=3, space="PSUM") as gpsum,
            tc.tile_pool(name="rqpsum", bufs=1, space="PSUM") as rqpsum,
        ):
            # own i-columns = slots 0..3 of xtb, first on the serialized DMA
            # path (gram slot 0 starts after the first half); small tables
            # follow in consumer order (exp bias/scale before vt).
            own_sb = singles.tile([128, NKC, IPC], f8)
            nc.sync.dma_start(out=own_sb[:, 0 : NKC // 2, :], in_=xtb[:, 0 : NKC // 2, 0:IPC])
            nc.sync.dma_start(out=own_sb[:, NKC // 2 : NKC, :], in_=xtb[:, NKC // 2 : NKC, 0:IPC])
            ftab_sb = singles.tile([128, NT + 2], fp32)
            nc.sync.dma_start(out=ftab_sb, in_=ftab[:])
            btab_sb = singles.tile([128, NT * C + 4 * C], bf16)
            # Stage small tiles through DVE so consumers wait on one
            # semaphore instead of the DMA queue fan-out (walrus caps the
            # per-instruction sync-wait count).
            qsq_s = singles.tile([128, NT], fp32)
            nc.vector.tensor_copy(qsq_s, ftab_sb[:, 0:NT])
            scl_s = singles.tile([128, 2], fp32)
            nc.vector.tensor_copy(scl_s, ftab_sb[:, NT : NT + 2])
            vt2_s = singles.tile([128, NT * C], bf16)
            vt1_s = singles.tile([128, 4 * C], bf16)
            # Dummy ACT op: loads the Exp table early and absorbs the DVE
            # wait so loop Exp ops only ever need the PE wait.
            warm = singles.tile([128, 2], fp32)
            nc.scalar.activation(warm, scl_s, Exp)

            # rq[q][i, b*C+cls] accumulates R_q over j for own block b
            rq = [rqpsum.tile([128, 4 * C], fp32, tag=f"rq{q}", name=f"rq{q}") for q in range(NQ)]

            def emit_weighted(g, es):
                # Flipped orientation: es block stationary, vt moving.
                # q-major, q=4 first so PE chases the DVE squaring chain.
                # PSUM zero-region semantics: exactly one start (first matmul
                # into the bank) and one stop (last) per rq tile.
                for q in range(NQ - 1, -1, -1):
                    for sl in range(4):
                        slot = 4 * g + sl
                        off, end = _slot_geom(g, sl)
                        eb = _ebase(g, sl)
                        for b in range(off, end):
                            if g == OWN_G and b == sl:
                                vtb = vt1_s[:, sl * C : (sl + 1) * C]
                            else:
                                vtb = vt2_s[:, slot * C : (slot + 1) * C]
                            col = eb + (b - off) * 128
                            nc.tensor.matmul(
                                rq[q][:, b * C : (b + 1) * C],
                                lhsT=es[q][:, col : col + 128],
                                rhs=vtb,
                                start=(g == 0 and sl == 0 and b == 0),
                                stop=(g == NGROUPS - 1 and sl == 3 and b == 3),
                            )

            pending = []
            for g in range(NGROUPS):
                if g == OWN_G:
                    wsrc = own_sb
                else:
                    wg = wpool.tile([128, NKC, 512], f8, tag="wg", name=f"w{g}")
                    src0 = g * 512
                    nc.sync.dma_start(out=wg, in_=xtb[:, :, src0 : src0 + 512])
                    if g == 1:
                        # vt tables land late (first consumer is weighted(0)
                        # at ~6us) so wg1/wg2 win the serialized DMA path
                        nc.sync.dma_start(out=btab_sb, in_=btab[:])
                        nc.vector.tensor_copy(vt2_s, btab_sb[:, 0 : NT * C])
                        nc.vector.tensor_copy(vt1_s, btab_sb[:, NT * C : NT * C + 4 * C])
                    wsrc = wg
                gw = 1280 if g == OWN_G else 1024  # total e-batch width
                es = {q: epool.tile([128, 1280], bf16, tag=f"e{q}", name=f"e{q}g{g}") for q in range(NQ)}
                for sl in (range(3, -1, -1) if g == OWN_G else range(4)):
                    slot = 4 * g + sl
                    off, end = _slot_geom(g, sl)
                    span = (end - off) * 128
                    eb = _ebase(g, sl)
                    gt = gpsum.tile([128, IPC], fp32, tag="g", name=f"g{slot}")
                    for m in range(NKP):
                        nc.tensor.matmul(
                            gt[:, 0:span],
                            lhsT=wsrc[:, 2 * m : 2 * m + 2, sl * 128 : (sl + 1) * 128],
                            rhs=own_sb[:, 2 * m : 2 * m + 2, off * 128 : end * 128],
                            start=(m == 0),
                            stop=(m == NKP - 1),
                            perf_mode=DR,
                        )
                    nc.scalar.activation(
                        es[4][:, eb : eb + span],
                        gt[:, 0:span],
                        Exp,
                        bias=qsq_s[:, slot : slot + 1],
                        scale=scl_s[:, 0:1],
                    )
                # squaring chain: e3/e2/e1 on DVE (2x_1p), e0 split between
                # the idle Pool engine and DVE
                nc.vector.tensor_mul(es[3][:, 0:gw], es[4][:, 0:gw], es[4][:, 0:gw])
                nc.vector.tensor_mul(es[2][:, 0:gw], es[3][:, 0:gw], es[3][:, 0:gw])
                nc.vector.tensor_mul(es[1][:, 0:gw], es[2][:, 0:gw], es[2][:, 0:gw])
                hw_ = 0 if g == NGROUPS - 1 else 13 * gw // 16
                if hw_:
                    nc.gpsimd.tensor_mul(es[0][:, 0:hw_], es[1][:, 0:hw_], es[1][:, 0:hw_])
                nc.vector.tensor_mul(es[0][:, hw_:gw], es[1][:, hw_:gw], es[1][:, hw_:gw])
                pending.append((g, es))
                if len(pending) > WLAG:
                    emit_weighted(*pending.pop(0))
            for item in pending:
                emit_weighted(*item)

            # tail: drain each rq as its last matmul lands (q=4 first);
            # copies alternate DVE/ACT; bulk DMA after q=1, final q=0 alone.
            stg = ostage.tile([128, NQ * 4 * C], fp32)
            for q in range(NQ - 1, -1, -1):
                dst = stg[:, q * 4 * C : (q + 1) * 4 * C]
                if q % 2 == 0:
                    nc.vector.tensor_copy(dst, rq[q])
                else:
                    nc.scalar.activation(dst, rq[q], Copy)
                if q == 2:
                    nc.sync.dma_start(out=rout[:, 8 * C :], in_=stg[:, 8 * C :])
            nc.sync.dma_start(out=rout[:, 0 : 8 * C], in_=stg[:, 0 : 8 * C])

    nc.compile()
    return nc


def _jseq(c):
    seq = list(range(4 * c, 4 * c + 4))
    for d in range(NCORES):
        if d == c:
            continue
        if d > c:
            seq += [4 * d, 4 * d + 1, 4 * d + 2, 4 * d + 3]
        else:
            seq += [4 * d + 2, 4 * d + 3, 4 * d, 4 * d + 1]
    return seq


def _prep(source, target, source_label, target_logits):
    X = np.concatenate([np.asarray(source), np.asarray(target)], axis=0)
    X64 = X.astype(np.float64)
    sq = np.einsum("nd,nd->n", X64, X64)
    colsum = X64.sum(axis=0)
    sum_l2 = 2.0 * N * sq.sum() - 2.0 * (colsum @ colsum)
    bw = sum_l2 / (N * N - N) / (2.0 ** (NQ // 2))
    cq = np.array([1.0 / (bw * 2.0**q) for q in range(NQ)])  # [5]

    sl = np.asarray(source_label, np.float64)
    tl = np.asarray(target_logits, np.float64)
    ssum = sl.sum(0)
    s_norm = np.where(ssum > 0, sl / np.where(ssum > 0, ssum, 1.0), 0.0)
    tsum = tl.sum(0)
    t_norm = np.where(tsum > 0, tl / np.where(tsum > 0, tsum, 1.0), 0.0)
    s_pres = np.zeros(C)
    np.add.at(s_pres, sl.argmax(1), 1.0)
    t_pres = np.zeros(C)
    np.add.at(t_pres, tl.argmax(1), 1.0)
    common = ((s_pres > 0) & (t_pres > 0)).astype(np.float64)
    V = np.concatenate([s_norm * common, -t_norm * common], axis=0)  # [N, C]

    # fp8 X^T in [p, k, jcol] layout (global j order; per-core slot perm later)
    X8 = X.astype(F8NP)                                   # [N, D]
    xt8 = np.ascontiguousarray(
        X8.T.reshape(NKC, 128, N).transpose(1, 0, 2)      # [128, 8, N]
    )
    Vb = V.astype(BFNP)
    sqt = sq.reshape(NT, 128)
    return X, sq, cq, V, Vb, xt8, sqt


def _core_inputs(c, cq, Vb, xt8, sqt):
    seq = _jseq(c)
    # xtb: permute j-tiles into slot order
    xtb = np.ascontiguousarray(
        xt8.reshape(128, NKC, NT, 128)[:, :, seq, :].reshape(128, NKC, NT * 128)
    )
    Vt = Vb.astype(np.float64).reshape(NT, 128, C)[seq]   # [NT, 128, C]
    vt2 = (2.0 * Vt).transpose(1, 0, 2).reshape(128, NT * C)
    vt1 = Vt[:4].transpose(1, 0, 2).reshape(128, 4 * C)
    btab = np.ascontiguousarray(np.concatenate([vt2, vt1], axis=1)).astype(BFNP)
    ftab = np.zeros((128, NT + 2), np.float32)
    ftab[:, 0:NT] = (-cq[4] * sqt[seq]).T
    ftab[:, NT] = 2.0 * cq[4]
    return {"xtb": xtb, "ftab": ftab, "btab": btab}


def _postprocess(results, sq, cq, V):
    # loss = 1/12 sum_q sum_i alpha_q[i] * (sum_cls V[i,cls] R_q[cls,i])
    loss = 0.0
    for c in range(NCORES):
        # r[p, q, b, cls] = R_q[cls, i] at i = 512c + 128b + p
        r = np.asarray(results[c]["r_out"], np.float64).reshape(128, NQ, 4, C)
        gi = c * IPC + np.arange(IPC)
        Vc = V[gi].reshape(4, 128, C)                     # [b, p, cls]
        alpha = np.exp(-np.outer(cq, sq[gi])).reshape(NQ, 4, 128)
        loss += np.einsum("qbp,bpc,pqbc->", alpha, Vc, r)
    return loss / C


def _run(in_maps, trace=False, **kw):
    global _BUILT
    if _BUILT is None:
        _BUILT = _build_program()
    return run_bass_kernel_spmd(_BUILT, in_maps, list(range(NCORES)), trace=trace, **kw)


def kernel(source, target, source_label, target_logits, _trace=False, _ret_bkr=False):
    X, sq, cq, V, Vb, xt8, sqt = _prep(source, target, source_label, target_logits)
    in_maps = [_core_inputs(c, cq, Vb, xt8, sqt) for c in range(NCORES)]
    try:
        bkr = _run(in_maps, trace=_trace)
    except Exception:
        # transient device wedge (NRT_EXEC_UNIT_UNRECOVERABLE) — retry once
        bkr = _run(in_maps, trace=_trace)
    loss = _postprocess(bkr.results, sq, cq, V)
    out = np.float32(loss)
    if _ret_bkr:
        return out, bkr
    return out
